# revision 23
# baseline (speedup 1.0000x reference)
"""Trainium2 Bass kernel for grouped-attention MoE routing.

Math (derived from the nn.Module):
  gate  = softmax(mlp(maxpool(conv(x))) + mlp(avgpool(conv(x))))      (B,45)
  sel   = sorted(top22(mean_b gate))                                  (22,)
  Per expert e with u = x[:, sel[e], :]:
    energy[l,m] = (a_e*u_l + g_e) * u_m   (rank-1; scalars a,g from weights)
    attn = softmax_m(energy);  s_l = sum_m u_m attn[l,m]
    y_l  = P_e*s_l + Q_e;      A[:,sel[e],:] = y * gate[:,sel[e]]
  G = x * A (flat);  return (G, A_flat)

Strategy: pure data parallel over batch on 8 cores; two launches with the
45-float routing reduction mediated on host (equivalent of the all-reduce).
"""

import numpy as np
from contextlib import ExitStack

import bass_rust
import concourse.bass as bass
import concourse.mybir as mybir
import concourse.tile as tile
from concourse.bass_utils import run_bass_kernel_spmd

_MULTIWAIT_OK = ("InstNoOp", "InstAllEngineBarrier",
                 "InstEventSemaphore", "InstUnconditionalBranch")


def legalize_sync_waits(nc):
    """walrus codegen on this stack rejects >1 sync wait on most
    instructions; hoist extra waits onto same-engine NoOps."""
    for func in nc.m.functions:
        for block in func.blocks:
            il = block.instructions
            out = []
            for inst in il:
                tname = type(inst).__name__
                si = getattr(inst, "sync_info", None)
                waits = list(si.on_wait) if si is not None else []
                if tname not in _MULTIWAIT_OK and len(waits) > 1:
                    for k, w in enumerate(waits):
                        nop = mybir.InstNoOp(
                            name=f"{inst.name}-synop{k}", ins=[], outs=[])
                        nop.engine = inst.engine
                        nop.sync_info = bass_rust.SyncInfo(
                            on_wait=[w], on_update=[])
                        out.append(nop)
                    inst.sync_info = bass_rust.SyncInfo(
                        on_wait=[], on_update=list(inst.sync_info.on_update))
                out.append(inst)
            il.clear()
            il.extend(out)

B, C, L, E = 8192, 45, 21, 22
NCORES = 8
BC = B // NCORES          # rows per core
P = 128                   # SBUF partitions
NT = BC // P              # batch tiles per core
CL = C * L                # 945
F32 = mybir.dt.float32
AF = mybir.ActivationFunctionType
ALU = mybir.AluOpType
AX = mybir.AxisListType

# channel groups for the gating conv matmul: 8 groups of <=6 channels
GROUPS = [list(range(g, min(g + 6, C))) for g in range(0, C, 6)]


def _ap(base, extra_free):
    """Custom free-dim access pattern on an SBUF tile slice.

    base: AP from tile[:, a:b]; extra_free: list of [step,count] replacing
    the free dims (partition dim kept)."""
    return bass.AP(tensor=base.tensor, offset=base.offset,
                   ap=[base.ap[0]] + extra_free)


def build_gate_program():
    nc = bass.Bass()
    x = nc.declare_dram_parameter("x", [BC, CL], F32, isOutput=False)
    # per-group block-diag gc_w^T (rows: (i,l) pairs), bias rows separate
    wblk = nc.declare_dram_parameter("wblk", [126, len(GROUPS) * 126], F32,
                                     isOutput=False)
    wbias = nc.declare_dram_parameter("wbias", [1, len(GROUPS) * 126], F32,
                                      isOutput=False)
    w1mx = nc.declare_dram_parameter("w1mx", [C, 25], F32, isOutput=False)
    w1av = nc.declare_dram_parameter("w1av", [C, 25], F32, isOutput=False)
    b1r = nc.declare_dram_parameter("b1r", [1, 25], F32, isOutput=False)
    w2 = nc.declare_dram_parameter("w2", [25, C], F32, isOutput=False)
    b2r = nc.declare_dram_parameter("b2r", [1, C], F32, isOutput=False)
    ident = nc.declare_dram_parameter("ident", [P, P], F32, isOutput=False)
    gate_o = nc.declare_dram_parameter("gate", [BC, C], F32, isOutput=True)
    gsum_o = nc.declare_dram_parameter("gsum", [C, 1], F32, isOutput=True)

    with tile.TileContext(nc) as tc, ExitStack() as ctx:
        singles = ctx.enter_context(tc.tile_pool(name="singles", bufs=1))
        xs = ctx.enter_context(tc.tile_pool(name="xs", bufs=2))
        work = ctx.enter_context(tc.tile_pool(name="work", bufs=3))
        small = ctx.enter_context(tc.tile_pool(name="small", bufs=4))
        ps = ctx.enter_context(tc.tile_pool(name="ps", bufs=2, space="PSUM"))
        psm = ctx.enter_context(tc.tile_pool(name="psm", bufs=1, space="PSUM"))
        pst = ctx.enter_context(tc.tile_pool(name="pst", bufs=1, space="PSUM"))
        pss = ctx.enter_context(tc.tile_pool(name="pss", bufs=1, space="PSUM"))

        # All PE-read tensors funnel through DVE so every matmul needs at
        # most one sync wait (fp32 self-loading matmul ISA limit).
        def dve_const(dram, p, n):
            raw = singles.tile([p, n], F32, name="raw_" + dram.name)
            nc.sync.dma_start(out=raw, in_=dram[:, :])
            t = singles.tile([p, n], F32, name="sb_" + dram.name)
            nc.vector.tensor_copy(out=t, in_=raw)
            return t

        sb_id = dve_const(ident, P, P)
        sb_wblk = dve_const(wblk, 126, len(GROUPS) * 126)
        sb_wbias = dve_const(wbias, 1, len(GROUPS) * 126)
        sb_w1mx = dve_const(w1mx, C, 25)
        sb_w1av = dve_const(w1av, C, 25)
        sb_b1r = dve_const(b1r, 1, 25)
        sb_w2 = dve_const(w2, 25, C)
        sb_b2r = dve_const(b2r, 1, C)
        ones_col = singles.tile([P, 1], F32)
        nc.vector.memset(ones_col, 1.0)
        ones_row = singles.tile([1, P], F32)
        nc.vector.memset(ones_row, 1.0)
        # dummy PE op: advances PE's observed DVE clock past the consts
        warm_ps = pss.tile([1, P], F32)
        nc.tensor.transpose(warm_ps, ones_col, sb_id)

        gsum_ps = pss.tile([C, 1], F32)

        def mlp_branch(h_sb, w1_sb):
            """h_sb (P,45) -> tanh((tanh(h@w1+b1))@w2+b2) as (P,45) SBUF."""
            hT_ps = psm.tile([C, P], F32, tag="mlpT")
            nc.tensor.transpose(hT_ps, h_sb, sb_id)
            hT = work.tile([C, P], F32, tag="hT_sb")
            nc.vector.tensor_copy(out=hT, in_=hT_ps)
            p1 = psm.tile([P, 25], F32, tag="mlpP")
            nc.tensor.matmul(p1, hT, w1_sb, start=True, stop=False)
            nc.tensor.matmul(p1, ones_row, sb_b1r, start=False, stop=True)
            p1c = small.tile([P, 25], F32, tag="p1c")
            nc.vector.tensor_copy(out=p1c, in_=p1)
            t1 = small.tile([P, 25], F32, tag="t1")
            nc.scalar.activation(out=t1, in_=p1c, func=AF.Tanh)
            t1d = small.tile([P, 25], F32, tag="t1d")
            nc.vector.tensor_copy(out=t1d, in_=t1)
            t1T_ps = psm.tile([25, P], F32, tag="mlpT")
            nc.tensor.transpose(t1T_ps, t1d, sb_id)
            t1T = work.tile([25, P], F32, tag="t1T_sb")
            nc.vector.tensor_copy(out=t1T, in_=t1T_ps)
            p2 = psm.tile([P, C], F32, tag="mlpP")
            nc.tensor.matmul(p2, t1T, sb_w2, start=True, stop=False)
            nc.tensor.matmul(p2, ones_row, sb_b2r, start=False, stop=True)
            p2c = small.tile([P, C], F32, tag="p2c")
            nc.vector.tensor_copy(out=p2c, in_=p2)
            z = small.tile([P, C], F32, tag="z")
            nc.scalar.activation(out=z, in_=p2c, func=AF.Tanh)
            return z

        for t in range(NT):
            xt = xs.tile([P, CL], F32)
            nc.sync.dma_start(out=xt, in_=x[t * P:(t + 1) * P, :])

            # conv: per channel-group transpose + block-diag matmul
            temp_ps = [pst.tile([P, 504], F32, tag="tempA", name="tempA"),
                       pst.tile([P, 504], F32, tag="tempB", name="tempB")]
            for g, chans in enumerate(GROUPS):
                w = len(chans) * L  # 126 or 63
                xT_ps = ps.tile([126, P], F32, tag="xT")
                nc.tensor.transpose(xT_ps[0:w, :],
                                    xt[:, chans[0] * L:chans[0] * L + w],
                                    sb_id)
                lhs = work.tile([126, P], F32, tag="lhs")
                nc.vector.tensor_copy(out=lhs[0:w, :], in_=xT_ps[0:w, :])
                half, slot = divmod(g, 4)
                dst = temp_ps[half][:, slot * 126:(slot + 1) * 126]
                nc.tensor.matmul(dst, lhs[0:w, :],
                                 sb_wblk[0:w, g * 126:(g + 1) * 126],
                                 start=True, stop=False)
                nc.tensor.matmul(dst, ones_row,
                                 sb_wbias[:, g * 126:(g + 1) * 126],
                                 start=False, stop=True)

            # mx/av pools over the 21 conv output channels
            mx = small.tile([P, 48], F32, tag="mx")
            av = small.tile([P, 48], F32, tag="av")
            for half in range(2):
                src = _ap(temp_ps[half][:, 0:504], [[126, 4], [21, 6], [1, L]])
                nc.vector.tensor_reduce(out=mx[:, half * 24:half * 24 + 24],
                                        in_=src, axis=AX.X, op=ALU.max)
                nc.vector.tensor_reduce(out=av[:, half * 24:half * 24 + 24],
                                        in_=src, axis=AX.X, op=ALU.add)

            zmx = mlp_branch(mx[:, 0:C], sb_w1mx)
            zav = mlp_branch(av[:, 0:C], sb_w1av)
            z = small.tile([P, C], F32, tag="zsum")
            nc.vector.tensor_add(out=z, in0=zmx, in1=zav)

            # softmax over the 45 channels
            m1 = small.tile([P, 1], F32, tag="m1")
            nc.vector.tensor_reduce(out=m1, in_=z, axis=AX.X, op=ALU.max)
            nm = small.tile([P, 1], F32, tag="nm")
            nc.vector.tensor_scalar_mul(out=nm, in0=m1, scalar1=-1.0)
            eg = small.tile([P, C], F32, tag="eg")
            ssum = small.tile([P, 1], F32, tag="ssum")
            nc.scalar.activation(out=eg, in_=z, func=AF.Exp, bias=nm,
                                 accum_out=ssum)
            rs = small.tile([P, 1], F32, tag="rs")
            nc.vector.reciprocal(out=rs, in_=ssum)
            gt = small.tile([P, C], F32, tag="gt")
            nc.vector.tensor_scalar_mul(out=gt, in0=eg, scalar1=rs)
            nc.sync.dma_start(out=gate_o[t * P:(t + 1) * P, :], in_=gt)

            nc.tensor.matmul(gsum_ps, gt, ones_col,
                             start=(t == 0), stop=(t == NT - 1))

        gs_sb = singles.tile([C, 1], F32)
        nc.vector.tensor_copy(out=gs_sb, in_=gsum_ps)
        nc.sync.dma_start(out=gsum_o[:, :], in_=gs_sb)
    legalize_sync_waits(nc)
    return nc


def build_attn_program(sel):
    """sel: sorted list of 22 selected channels (python ints, baked in)."""
    # runs of consecutive channels -> contiguous slices in both x and expert idx
    runs = []  # (chan0, e0, len)
    i = 0
    while i < E:
        j = i
        while j + 1 < E and sel[j + 1] == sel[j] + 1:
            j += 1
        runs.append((sel[i], i, j - i + 1))
        i = j + 1

    EL = E * L            # 462
    ELM = E * L * L       # 9702

    nc = bass.Bass()
    x = nc.declare_dram_parameter("x", [BC, CL], F32, isOutput=False)
    gsel = nc.declare_dram_parameter("gsel", [BC, E], F32, isOutput=False)
    avec = nc.declare_dram_parameter("avec", [EL], F32, isOutput=False)
    gvec = nc.declare_dram_parameter("gvec", [EL], F32, isOutput=False)
    pvec = nc.declare_dram_parameter("pvec", [E], F32, isOutput=False)
    qvec = nc.declare_dram_parameter("qvec", [E], F32, isOutput=False)
    a_o = nc.declare_dram_parameter("asel", [BC, EL], F32, isOutput=True)
    g_o = nc.declare_dram_parameter("gout", [BC, EL], F32, isOutput=True)

    with tile.TileContext(nc) as tc, ExitStack() as ctx:
        singles = ctx.enter_context(tc.tile_pool(name="singles", bufs=1))
        xs = ctx.enter_context(tc.tile_pool(name="xs", bufs=2))
        big = ctx.enter_context(tc.tile_pool(name="big", bufs=1))
        mid = ctx.enter_context(tc.tile_pool(name="mid", bufs=2))
        outs = ctx.enter_context(tc.tile_pool(name="outs", bufs=2))

        def bconst(dram, n):
            base = dram[:]
            t = singles.tile([P, n], F32, name="bc_" + dram.name)
            nc.gpsimd.dma_start(
                out=t, in_=bass.AP(tensor=base.tensor, offset=base.offset,
                                   ap=[[0, P], [1, n]]))
            return t

        aB = bconst(avec, EL)
        gB = bconst(gvec, EL)
        pB = bconst(pvec, E)
        qB = bconst(qvec, E)

        for t in range(NT):
            xt = xs.tile([P, CL], F32)
            nc.sync.dma_start(out=xt, in_=x[t * P:(t + 1) * P, :])
            gs = xs.tile([P, E], F32, tag="gs")
            nc.sync.dma_start(out=gs, in_=gsel[t * P:(t + 1) * P, :])

            # gather the 22 selected channels once; all later ops contiguous
            u = mid.tile([P, EL], F32, tag="u")
            for (c0, e0, n) in runs:
                nc.vector.tensor_copy(out=u[:, e0 * L:(e0 + n) * L],
                                      in_=xt[:, c0 * L:(c0 + n) * L])

            # kappa[b,(e,l)] = a_e * u[b,e,l] + g_e
            kap = mid.tile([P, EL], F32, tag="kap")
            nc.vector.tensor_mul(out=kap, in0=u, in1=aB)
            nc.vector.tensor_add(out=kap, in0=kap, in1=gB)

            # energy[b,(e,l,m)] = kappa[b,e,l] * u[b,e,m]; exp in place
            en = big.tile([P, ELM], F32, tag="en")
            en3 = _ap(en[:, 0:ELM], [[L * L, E], [L, L], [1, L]])
            kap_lrep = _ap(kap[:, 0:EL], [[L, E], [1, L], [0, L]])
            u_mrep = _ap(u[:, 0:EL], [[L, E], [0, L], [1, L]])
            nc.vector.tensor_mul(out=en3, in0=kap_lrep, in1=u_mrep)
            nc.scalar.activation(out=en, in_=en, func=AF.Exp)

            den = mid.tile([P, EL], F32, tag="den")
            nc.vector.tensor_reduce(
                out=den, in_=_ap(en[:, 0:ELM], [[L, EL], [1, L]]),
                axis=AX.X, op=ALU.add)

            # en <- en * u_m (numerator weights), then reduce
            nc.vector.tensor_mul(out=en3, in0=en3, in1=u_mrep)
            num = mid.tile([P, EL], F32, tag="num")
            nc.vector.tensor_reduce(
                out=num, in_=_ap(en[:, 0:ELM], [[L, EL], [1, L]]),
                axis=AX.X, op=ALU.add)

            nc.vector.reciprocal(out=den, in_=den)
            nc.vector.tensor_mul(out=num, in0=num, in1=den)  # s

            # A = s * (gate*P)_rep + (gate*Q)_rep ; G = A * u
            gp = mid.tile([P, E], F32, tag="gp")
            nc.vector.tensor_mul(out=gp, in0=gs, in1=pB)
            gq = mid.tile([P, E], F32, tag="gq")
            nc.vector.tensor_mul(out=gq, in0=gs, in1=qB)
            at = outs.tile([P, EL], F32, tag="at")
            nc.vector.tensor_mul(out=_ap(at[:, 0:EL], [[L, E], [1, L]]),
                                 in0=_ap(num[:, 0:EL], [[L, E], [1, L]]),
                                 in1=_ap(gp[:, 0:E], [[1, E], [0, L]]))
            nc.vector.tensor_add(out=_ap(at[:, 0:EL], [[L, E], [1, L]]),
                                 in0=_ap(at[:, 0:EL], [[L, E], [1, L]]),
                                 in1=_ap(gq[:, 0:E], [[1, E], [0, L]]))
            gt = outs.tile([P, EL], F32, tag="gt")
            nc.vector.tensor_mul(out=gt, in0=at, in1=u)
            nc.sync.dma_start(out=a_o[t * P:(t + 1) * P, :], in_=at)
            nc.sync.dma_start(out=g_o[t * P:(t + 1) * P, :], in_=gt)
    legalize_sync_waits(nc)
    return nc


def _host_params(inputs):
    gc_w, gc_b = inputs["gc_w"], inputs["gc_b"]
    ng = len(GROUPS)
    wblk = np.zeros((126, ng * 126), np.float32)
    wbias = np.zeros((1, ng * 126), np.float32)
    for g, chans in enumerate(GROUPS):
        for k, _ in enumerate(chans):
            c0 = g * 126 + k * L
            wblk[k * L:(k + 1) * L, c0:c0 + L] = gc_w.T
            wbias[0, c0:c0 + L] = gc_b
    w1mx = inputs["w1"].T.astype(np.float32)
    w1av = (inputs["w1"].T / L).astype(np.float32)
    b1r = inputs["b1"][None, :].astype(np.float32)
    w2 = inputs["w2"].T.astype(np.float32)
    b2r = inputs["b2"][None, :].astype(np.float32)
    return wblk, wbias, w1mx, w1av, b1r, w2, b2r


_CACHE = {}


def kernel(**inputs):
    inputs = {k: np.ascontiguousarray(np.asarray(v)) for k, v in inputs.items()}
    x = inputs["x"].astype(np.float32).reshape(B, CL)
    wblk, wbias, w1mx, w1av, b1r, w2, b2r = _host_params(inputs)
    ident = np.eye(P, dtype=np.float32)
    cores = list(range(NCORES))

    if "gate" not in _CACHE:
        _CACHE["gate"] = build_gate_program()
    nc1 = _CACHE["gate"]
    maps1 = [{"x": x[i * BC:(i + 1) * BC], "wblk": wblk, "wbias": wbias,
              "w1mx": w1mx, "w1av": w1av, "b1r": b1r, "w2": w2, "b2r": b2r,
              "ident": ident} for i in cores]
    r1 = run_bass_kernel_spmd(nc1, maps1, cores).results
    gate = np.concatenate([r["gate"] for r in r1], 0)          # (B,45)
    mean_gate = np.sum([r["gsum"][:, 0] for r in r1], 0) / B   # (45,)
    sel = np.sort(np.argsort(-mean_gate, kind="stable")[:E])

    wq, bq = inputs["wq"], inputs["bq"]
    wk, bk = inputs["wk"], inputs["bk"]
    wv, bv = inputs["wv"], inputs["bv"]
    wo, bo = inputs["wo"], inputs["bo"]
    alpha = (wq * wk).sum(1).astype(np.float32)
    gamma = (bq * wk).sum(1).astype(np.float32)
    pv = (wo * wv).sum(1).astype(np.float32)
    qv = ((wo * bv).sum(1) + bo).astype(np.float32)
    avec = np.repeat(alpha, L)
    gvec = np.repeat(gamma, L)
    gsel = np.ascontiguousarray(gate[:, sel])

    key = tuple(sel.tolist())
    if _CACHE.get("attn_key") != key:
        _CACHE["attn"] = build_attn_program([int(s) for s in sel])
        _CACHE["attn_key"] = key
    nc2 = _CACHE["attn"]
    maps2 = [{"x": x[i * BC:(i + 1) * BC], "gsel": gsel[i * BC:(i + 1) * BC],
              "avec": avec, "gvec": gvec, "pvec": pv, "qvec": qv}
             for i in cores]
    r2 = run_bass_kernel_spmd(nc2, maps2, cores).results
    asel = np.concatenate([r["asel"] for r in r2], 0)          # (B,462)
    gout = np.concatenate([r["gout"] for r in r2], 0)

    cols = (np.repeat(sel * L, L) + np.tile(np.arange(L), E))  # (462,)
    A_full = np.zeros((B, CL), np.float32)
    G_full = np.zeros((B, CL), np.float32)
    A_full[:, cols] = asel
    G_full[:, cols] = gout
    return G_full, A_full
